# revision 47
# baseline (speedup 1.0000x reference)
"""AdapterLayer (LN -> down-proj -> ReLU -> up-proj -> residual) on 8 TRN2 NeuronCores.

Sharding: pure data-parallel over the 16384 tokens (2048 tokens/core); adapter
params are replicated (tiny). No collectives.

The kernel is PE-bound (~500 matmuls of 512 rows/core = 109us at 2.4 GHz), and
the TRN2 PE only reaches its 2.4 GHz p-state after ~3us of *continuous*
execution, so the whole design aims at a gap-free PE instruction stream.

Key idea: DEFERRED LAYERNORM. Instead of standardizing x before the down
projection (which puts DVE stats + ScalarE apply on the PE's critical path),
the down-proj consumes a plain bf16 cast of RAW x, and LN is folded in after
the matmul, exactly:

    W_eff @ ((x - mu) * istd) = istd * (W_eff @ x) - (istd * mu) * w1,
    w1 = W_eff @ ones_D

  - PSUM accumulates  W_eff @ x_raw  plus a K=2 rank-1 seed matmul
    [std; -mu]^T @ [b_eff; w1]  (std = 1/istd), and the ReLU activation
    applies the per-token istd scale:  relu(istd * psum) = relu(down).
  - The PE's input chain is x-DMA -> bf16 cast (ScalarE) -> xbar transpose
    (SyncE): no stats dependency. bn_stats/aggr run on DVE in parallel and
    are only needed by the seed matmul / ReLU, ~2 pipeline phases later.
  - The [std; -mu] column pair is row-ified for the seed via a tiny PE
    transpose (128 cycles) against a host-provided identity.

Per-tile dataflow (128 tokens):
  a1: DMA x [128,2048] f32 (ScalarE HWDGE) -> bf16 cast (ScalarE) ->
      xbar-transpose xT -> [d, tok] slabs (SyncE, transpose-pure).
      In parallel: bn_stats/bn_aggr (DVE), std=sqrt(var+eps) (ScalarE),
      istd=1/std (DVE), sm=[std, -mu] bf16 -> PE-transpose -> S [2,128].
      x += b_up on GpSimd (residual bias, off every critical path).
  a2: down-proj (K=2048 over 16 slabs, stationary = xT slab) + K=2 LN seed
      -> relu(istd * psum) on ScalarE to bf16 -> xbar-transpose rdT.
  b:  up-proj in 4 [128,512] quarters (K=512 over 4 slabs) -> DVE residual
      add (psum + x, x already carries b_up) -> DMA out (GpSimd SWDGE).

Engine roles keep PE-dependent ops off the a1 feed path:
  ScalarE: x-issue, cast, sqrt, relu    DVE: stats, istd, residual adds
  SyncE:   xbar transposes only         GpSimd: x+=b_up, out-issue
Every DMA chains behind the DMA ~10 positions earlier through a recycled
completion-semaphore ring, so all DMAs are kept few and fast.

ln_gamma/ln_beta are folded on the host into W_eff/b_eff. Matmul operands
are bf16 (f32 accumulation); LN stats and the residual add stay f32.
"""

import numpy as np
import ml_dtypes

import concourse.bass as bass
import concourse.tile as tile
from concourse import mybir

from concourse.bass_utils import run_bass_kernel_spmd

# ---------------------------------------------------------------------------
# Workaround: the pinned walrus rejects >2 sem-waits on one instruction, but
# Tile's tail drain aggregates a wait per outstanding semaphore. Split them
# into one-wait-per-nop on the sync engine ahead of the drain.
from concourse.tile_sem_assignment import N_PROCS
from bass_rust import VectorClock, ScopedClock


def _drain_and_barrier_split(self, tick_clock, wait_clock):
    gc = tick_clock.global_clock
    for p in range(N_PROCS):
        if gc[p] == 0:
            continue
        c = [0] * N_PROCS
        c[p] = gc[p]
        nop = self.nc.sync.nop(nofuse=True, hint=f"drain_wait_p{p}")
        wait_clock.add_sem_waits(nop.ins, ScopedClock({None: VectorClock(c)}))
    self.nc.sync.drain()
    self.nc.all_engine_barrier()
    assert self.sems is not None
    popped = self.nc._tile_sem_poison_stack.pop()
    assert popped is self._sem_poison
    self.nc.clear_and_free_semaphores(list(self.sems.allocated().values()))
    self.nc.all_engine_barrier()


tile.TileContext._drain_and_barrier = _drain_and_barrier_split

# Same walrus limitation mid-kernel: any scheduled instruction may carry at
# most 2 sem-waits. Split excess waits onto same-engine NoOps committed just
# ahead of the instruction.
import bass_rust as _bass_rust

_MAX_WAITS = 1
_orig_commit = tile.TileContext._commit_instruction
_wsplit_counter = [0]


def _commit_instruction_split(self, inst, lazy_reg_writes=True):
    si = inst.sync_info
    if (
        si is not None
        and si.on_wait
        and len(si.on_wait) > _MAX_WAITS
        and inst.engine != mybir.EngineType.Unassigned
    ):
        waits = list(si.on_wait)
        keep = waits[-_MAX_WAITS:]
        extra = waits[:-_MAX_WAITS]
        for i in range(0, len(extra), _MAX_WAITS):
            _wsplit_counter[0] += 1
            nop = _bass_rust.InstNoOp(
                name=f"wsplit-{_wsplit_counter[0]}", ins=[], outs=[]
            )
            nop.engine = inst.engine
            nop.sync_info = _bass_rust.SyncInfo(
                on_wait=extra[i:i + _MAX_WAITS], on_update=[]
            )
            self._add_instruction(nop)
        inst.sync_info = _bass_rust.SyncInfo(
            on_wait=keep, on_update=list(si.on_update)
        )
    return _orig_commit(self, inst, lazy_reg_writes)


tile.TileContext._commit_instruction = _commit_instruction_split
# ---------------------------------------------------------------------------

B, S, D, H = 4, 4096, 2048, 512
EPS = 1e-5
NCORES = 8
TOK = B * S // NCORES  # tokens per core
P = 128
NT = TOK // P          # 16 token tiles per core
KC = D // P            # 16 contraction chunks for down-proj
HC = H // P            # 4 contraction chunks for up-proj
LAG2 = 3               # a2 trails a1 by this many tiles
LAG3 = 4               # b trails a1 by this many tiles

F32 = mybir.dt.float32
BF16 = mybir.dt.bfloat16
FP8 = mybir.dt.float8e4


def build_nc():
    nc = bass.Bass("TRN2", target_bir_lowering=False, debug=False, num_devices=NCORES)

    x_ext = nc.declare_dram_parameter("x", [TOK, D], F32, isOutput=False)
    wdT_ext = nc.declare_dram_parameter("wdT", [P, KC, H], FP8, isOutput=False)
    wuT_ext = nc.declare_dram_parameter("wuT", [P, HC, D], BF16, isOutput=False)
    seedC_ext = nc.declare_dram_parameter("seedC", [2, H], BF16, isOutput=False)
    id128_ext = nc.declare_dram_parameter("id128", [P, P], BF16, isOutput=False)
    id128f8_ext = nc.declare_dram_parameter("id128f8", [P, P], FP8, isOutput=False)
    bu_ext = nc.declare_dram_parameter("bu", [1, D], BF16, isOutput=False)
    out_ext = nc.declare_dram_parameter("out", [TOK, D], F32, isOutput=True)

    with tile.TileContext(nc) as tc:
        with (
            tc.tile_pool(name="singles", bufs=1) as singles,
            tc.tile_pool(name="xp", bufs=6) as xp,
            tc.tile_pool(name="statp", bufs=12) as statp,
            tc.tile_pool(name="yp", bufs=3) as yp,
            tc.tile_pool(name="ytp", bufs=5) as ytp,
            tc.tile_pool(name="rp", bufs=3) as rp,
            tc.tile_pool(name="rtp", bufs=4) as rtp,
            tc.tile_pool(name="op", bufs=3) as op,
            tc.tile_pool(name="pdp", bufs=2, space="PSUM") as pdp,
            tc.tile_pool(name="pup", bufs=3, space="PSUM") as pup,
            tc.tile_pool(name="ptY", bufs=2, space="PSUM") as ptY,
            tc.tile_pool(name="ptR", bufs=1, space="PSUM") as ptR,
        ):
            # -------- persistent tiles ------------------------------------
            # DMA traffic is segregated by class: bulk HBM transfers (x-in,
            # out, wdT) ride the GpSimd SWDGE ring; the latency-critical
            # xbar transposes get the 8-semaphore HWDGE ring essentially to
            # themselves (only these few small/fast startup loads precede
            # them there), so no transpose ever chains behind a 1MB x
            # transfer through a recycled completion semaphore.
            bu_row = singles.tile([1, D], BF16)
            nc.scalar.dma_start(bu_row[:], bu_ext[:])
            seedC = singles.tile([2, H], BF16)
            nc.scalar.dma_start(seedC[:], seedC_ext[:])
            id128 = singles.tile([P, P], BF16)
            nc.scalar.dma_start(id128[:], id128_ext[:])
            id128f8 = singles.tile([P, P], FP8)
            nc.scalar.dma_start(id128f8[:], id128f8_ext[:])
            wuT = singles.tile([P, HC, D], BF16)  # loaded after wdT, below
            wdT = singles.tile([P, KC, H], FP8)   # loaded after a1(0), below
            ones_row = singles.tile([1, P], BF16)
            nc.vector.memset(ones_row[:], 1.0)
            epst = singles.tile([P, 1], F32)
            nc.vector.memset(epst[:], EPS)

            # Broadcast b_up across partitions ON DEVICE (rank-1 K=1 matmuls
            # + PSUM->SBUF copies) instead of a descriptor-heavy duplicated
            # DRAM read: the PE is idle at startup, the DMA ring is not.
            bu_bc = singles.tile([P, D], F32)
            for q in range(4):
                pq = pup.tile([P, 512], F32)
                nc.tensor.matmul(pq[:], ones_row[:],
                                 bu_row[:, q * 512:(q + 1) * 512],
                                 start=True, stop=True)
                nc.scalar.copy(bu_bc[:, q * 512:(q + 1) * 512], pq[:])

            def phase_a1(t):
                """Load x; cast+transpose feed the PE; stats in parallel."""
                x_sb = xp.tile([P, D], F32)
                y_sb = yp.tile([P, D], BF16)
                yT = ytp.tile([P, KC, P], FP8)
                # x-in must ride the (fast) scalar HWDGE queue: SWDGE
                # transfer bandwidth can't keep the feed ahead of the PE.
                nc.scalar.dma_start(x_sb[:], x_ext[t * P:(t + 1) * P, :])
                # Priority keeps the cast ahead of bulk scalar work, but a2's
                # relu/rdT-copy (same priority class, emitted earlier in the
                # iteration) stay in front of it in ScalarE's FIFO.
                with tc.high_priority():
                    nc.scalar.activation(
                        y_sb[:], x_sb[:],
                        mybir.ActivationFunctionType.Identity,
                    )
                # Transpose y on the PE itself (128 cycles/slab against the
                # identity): the PE paces its own input, so no transpose can
                # ever chain behind a bulk transfer through the recycled DMA
                # completion semaphores. Slabs stage in PSUM (8 per bank,
                # each slab its own start/stop write) and copy back to SBUF
                # on ScalarE (group 0) / DVE (group 1).
                # (transposes run in bf16 -- fp8 PE-transpose needs a
                # stride-2 output -- and the PSUM->SBUF copy casts to fp8)
                for g in range(2):
                    pt = ptY.tile([P, 8, P], BF16)
                    for j in range(8):
                        nc.tensor.transpose(
                            pt[:, j, :],
                            y_sb[:, (8 * g + j) * P:(8 * g + j + 1) * P],
                            id128[:])
                    if g == 0:
                        nc.scalar.copy(yT[:, 0:8, :], pt[:])
                    else:
                        nc.vector.tensor_scalar_add(yT[:, 8:16, :], pt[:], 0.0)

                # LN stats: mu/var on DVE, std=sqrt(var+eps), istd=1/std.
                # Only consumed by the seed matmul / ReLU ~2 phases later.
                # stats on the bf16 cast: 2x DVE rate, and the seed/ReLU
                # consumers are ~2 phases away so the cast dependency is free
                st = statp.tile([P, 4, 6], F32)
                for i in range(4):
                    nc.vector.bn_stats(st[:, i, :], y_sb[:, i * 512:(i + 1) * 512])
                mv = statp.tile([P, 2], F32)
                nc.vector.bn_aggr(mv[:], st[:])
                std = statp.tile([P, 1], F32)
                nc.scalar.activation(
                    std[:], mv[:, 1:2], mybir.ActivationFunctionType.Sqrt,
                    bias=epst[:], scale=1.0,
                )
                istd = statp.tile([P, 1], F32)
                nc.vector.reciprocal(istd[:], std[:])
                # sm = [std, -mu] as bf16 columns; row-ified in a2 via a tiny
                # PE transpose (the stats have ~3 tiles of slack until then).
                sm = statp.tile([P, 2], BF16)
                nc.scalar.copy(sm[:, 0:1], std[:])
                nc.vector.tensor_scalar(
                    sm[:, 1:2], mv[:, 0:1], -1.0, None, mybir.AluOpType.mult)

                # fold b_up into the residual on GpSimd: x <- x + b_up (after
                # the cast and bn_stats consumed raw x).
                nc.gpsimd.tensor_add(x_sb[:], x_sb[:], bu_bc[:])
                return x_sb, yT, sm, istd

            def phase_a2(t, x_sb, yT, sm, istd):
                """Down-proj on raw xT + K=2 LN seed, relu(istd*psum), rdT."""
                # One PSUM bank stages both the [std;-mu] row pair (slab 4)
                # and the 4 relu'd-down slabs (slabs 0-3) for this tile.
                pt = ptR.tile([P, 5, P], BF16)
                nc.tensor.transpose(pt[0:2, 4, :], sm[:], id128[:])
                srow = statp.tile([2, P], BF16)
                with tc.high_priority():
                    nc.scalar.copy(srow[:], pt[0:2, 4, :])

                # fp8e4m3 DoubleRow: two K=128 slabs per instruction at 2x
                # rate; yT/wdT's [P, KC, free] layout is already the required
                # [K, 2, free] sub-slab form.
                pd = pdp.tile([P, H], F32)
                for k in range(0, KC, 2):
                    nc.tensor.matmul(pd[:], yT[:, k:k + 2, :], wdT[:, k:k + 2, :],
                                     start=(k == 0), stop=False,
                                     perf_mode=mybir.MatmulPerfMode.DoubleRow)
                # LN correction seed: [std;-mu]^T @ [b_eff;w1], last so the
                # stats chain has the whole down-proj's duration of slack.
                nc.tensor.matmul(pd[:], srow[:], seedC[:], start=False, stop=True)

                rd = rp.tile([P, H], BF16)
                with tc.high_priority():
                    nc.scalar.activation(rd[:], pd[:],
                                         mybir.ActivationFunctionType.Relu,
                                         scale=istd[:])
                for c in range(HC):
                    nc.tensor.transpose(pt[:, c, :], rd[:, c * P:(c + 1) * P],
                                        id128[:])
                rdT = rtp.tile([P, HC, P], BF16)
                with tc.high_priority():
                    nc.scalar.copy(rdT[:], pt[:, 0:HC, :])
                return x_sb, rdT

            def phase_b(t, x_sb, rdT):
                """Up-proj + residual add (x already carries b_up) + store."""
                o_sb = op.tile([P, D], F32)
                for q in range(4):
                    pq = pup.tile([P, 512], F32)
                    n0 = q * 512
                    for c in range(HC):
                        nc.tensor.matmul(
                            pq[:], rdT[:, c, :], wuT[:, c, n0:n0 + 512],
                            start=(c == 0), stop=(c == HC - 1),
                        )
                    sl = slice(n0, n0 + 512)
                    nc.vector.tensor_add(o_sb[:, sl], pq[:], x_sb[:, sl])

                if t == NT - 1:
                    # split the last store across both queues: it is the
                    # only transfer left on the drain path.
                    nc.gpsimd.dma_start(out_ext[t * P:(t + 1) * P, 0:1024],
                                        o_sb[:, 0:1024])
                    nc.scalar.dma_start(out_ext[t * P:(t + 1) * P, 1024:D],
                                        o_sb[:, 1024:D])
                else:
                    nc.gpsimd.dma_start(out_ext[t * P:(t + 1) * P, :], o_sb[:])

            # staggered software pipeline: a1 runs LAG2 tiles ahead of a2 and
            # LAG3 ahead of b. Tile 0's x-load is emitted BEFORE the bulk
            # wdT loads so it leads the SWDGE queue; the k-chunked wdT lands
            # slab-by-slab just ahead of the first down-proj matmuls.
            h1, h2_ = {}, {}
            h1[0] = phase_a1(0)
            for k in range(0, KC, 4):
                nc.scalar.dma_start(wdT[:, k:k + 4, :], wdT_ext[:, k:k + 4, :])
            nc.scalar.dma_start(wuT[:, 0:2, :], wuT_ext[:, 0:2, :])
            nc.scalar.dma_start(wuT[:, 2:4, :], wuT_ext[:, 2:4, :])
            for t in range(1, NT + LAG3):
                if LAG2 <= t < NT + LAG2:
                    h2_[t - LAG2] = phase_a2(t - LAG2, *h1.pop(t - LAG2))
                if LAG3 <= t:
                    phase_b(t - LAG3, *h2_.pop(t - LAG3))
                if t < NT:
                    h1[t] = phase_a1(t)

    return nc


_NC_CACHE = None


def _get_nc():
    global _NC_CACHE
    if _NC_CACHE is None:
        _NC_CACHE = build_nc()
    return _NC_CACHE


def make_in_maps(x, ln_gamma, ln_beta, W_down, b_down, W_up, b_up):
    x2d = np.ascontiguousarray(np.asarray(x, dtype=np.float32).reshape(B * S, D))

    # Fold LN affine (gamma/beta) into the down projection exactly:
    #   W_down @ (yhat*gamma + beta) = (W_down*gamma) @ yhat + W_down @ beta
    Wd = np.asarray(W_down, dtype=np.float64)
    gamma = np.asarray(ln_gamma, dtype=np.float64)
    beta = np.asarray(ln_beta, dtype=np.float64)
    wd_eff = Wd * gamma[None, :]
    bd_eff = np.asarray(b_down, dtype=np.float64) + Wd @ beta
    w1 = wd_eff.sum(axis=1)  # W_eff @ ones_D, for the deferred-LN correction

    bf = ml_dtypes.bfloat16
    f8 = ml_dtypes.float8_e4m3fn
    wdT_host = np.ascontiguousarray(
        wd_eff.T.reshape(KC, P, H).transpose(1, 0, 2)).astype(f8)
    wuT_host = np.ascontiguousarray(
        np.asarray(W_up, dtype=np.float64).T.reshape(HC, P, D).transpose(1, 0, 2)
    ).astype(bf)
    seedC_host = np.ascontiguousarray(
        np.stack([bd_eff, w1]).reshape(2, H)).astype(bf)
    id128_host = np.eye(P, dtype=np.float32).astype(bf)
    id128f8_host = np.eye(P, dtype=np.float32).astype(f8)
    bu_host = np.ascontiguousarray(
        np.asarray(b_up, dtype=np.float32).reshape(1, D)).astype(bf)

    in_maps = []
    for i in range(NCORES):
        in_maps.append({
            "x": x2d[i * TOK:(i + 1) * TOK],
            "wdT": wdT_host,
            "wuT": wuT_host,
            "seedC": seedC_host,
            "id128": id128_host,
            "id128f8": id128f8_host,
            "bu": bu_host,
        })
    return in_maps


def gather_out(results):
    return np.concatenate(
        [np.asarray(results[i]["out"], dtype=np.float32) for i in range(NCORES)],
        axis=0,
    ).reshape(B, S, D)


def kernel(x, ln_gamma, ln_beta, W_down, b_down, W_up, b_up):
    nc = _get_nc()
    in_maps = make_in_maps(x, ln_gamma, ln_beta, W_down, b_down, W_up, b_up)
    res = run_bass_kernel_spmd(nc, in_maps, core_ids=list(range(NCORES)))
    return gather_out(res.results)


# revision 48
# speedup vs baseline: 1.1660x; 1.1660x over previous
"""AdapterLayer (LN -> down-proj -> ReLU -> up-proj -> residual) on 8 TRN2 NeuronCores.

Sharding: pure data-parallel over the 16384 tokens (2048 tokens/core); adapter
params are replicated (tiny). No collectives.

The kernel is PE-bound (~500 matmuls of 512 rows/core = 109us at 2.4 GHz), and
the TRN2 PE only reaches its 2.4 GHz p-state after ~3us of *continuous*
execution, so the whole design aims at a gap-free PE instruction stream.

Key idea: DEFERRED LAYERNORM. Instead of standardizing x before the down
projection (which puts DVE stats + ScalarE apply on the PE's critical path),
the down-proj consumes a plain bf16 cast of RAW x, and LN is folded in after
the matmul, exactly:

    W_eff @ ((x - mu) * istd) = istd * (W_eff @ x) - (istd * mu) * w1,
    w1 = W_eff @ ones_D

  - PSUM accumulates  W_eff @ x_raw  plus a K=2 rank-1 seed matmul
    [std; -mu]^T @ [b_eff; w1]  (std = 1/istd), and the ReLU activation
    applies the per-token istd scale:  relu(istd * psum) = relu(down).
  - The PE's input chain is x-DMA -> bf16 cast (ScalarE) -> xbar transpose
    (SyncE): no stats dependency. bn_stats/aggr run on DVE in parallel and
    are only needed by the seed matmul / ReLU, ~2 pipeline phases later.
  - The [std; -mu] column pair is row-ified for the seed via a tiny PE
    transpose (128 cycles) against a host-provided identity.

Per-tile dataflow (128 tokens):
  a1: DMA x [128,2048] f32 (ScalarE HWDGE) -> bf16 cast (ScalarE) ->
      xbar-transpose xT -> [d, tok] slabs (SyncE, transpose-pure).
      In parallel: bn_stats/bn_aggr (DVE), std=sqrt(var+eps) (ScalarE),
      istd=1/std (DVE), sm=[std, -mu] bf16 -> PE-transpose -> S [2,128].
      x += b_up on GpSimd (residual bias, off every critical path).
  a2: down-proj (K=2048 over 16 slabs, stationary = xT slab) + K=2 LN seed
      -> relu(istd * psum) on ScalarE to bf16 -> xbar-transpose rdT.
  b:  up-proj in 4 [128,512] quarters (K=512 over 4 slabs) -> DVE residual
      add (psum + x, x already carries b_up) -> DMA out (GpSimd SWDGE).

Engine roles keep PE-dependent ops off the a1 feed path:
  ScalarE: x-issue, cast, sqrt, relu    DVE: stats, istd, residual adds
  SyncE:   xbar transposes only         GpSimd: x+=b_up, out-issue
Every DMA chains behind the DMA ~10 positions earlier through a recycled
completion-semaphore ring, so all DMAs are kept few and fast.

ln_gamma/ln_beta are folded on the host into W_eff/b_eff. Matmul operands
are bf16 (f32 accumulation); LN stats and the residual add stay f32.
"""

import numpy as np
import ml_dtypes

import concourse.bass as bass
import concourse.tile as tile
from concourse import mybir

from concourse.bass_utils import run_bass_kernel_spmd

# ---------------------------------------------------------------------------
# Workaround: the pinned walrus rejects >2 sem-waits on one instruction, but
# Tile's tail drain aggregates a wait per outstanding semaphore. Split them
# into one-wait-per-nop on the sync engine ahead of the drain.
from concourse.tile_sem_assignment import N_PROCS
from bass_rust import VectorClock, ScopedClock


def _drain_and_barrier_split(self, tick_clock, wait_clock):
    gc = tick_clock.global_clock
    for p in range(N_PROCS):
        if gc[p] == 0:
            continue
        c = [0] * N_PROCS
        c[p] = gc[p]
        nop = self.nc.sync.nop(nofuse=True, hint=f"drain_wait_p{p}")
        wait_clock.add_sem_waits(nop.ins, ScopedClock({None: VectorClock(c)}))
    self.nc.sync.drain()
    self.nc.all_engine_barrier()
    assert self.sems is not None
    popped = self.nc._tile_sem_poison_stack.pop()
    assert popped is self._sem_poison
    self.nc.clear_and_free_semaphores(list(self.sems.allocated().values()))
    self.nc.all_engine_barrier()


tile.TileContext._drain_and_barrier = _drain_and_barrier_split

# Same walrus limitation mid-kernel: any scheduled instruction may carry at
# most 2 sem-waits. Split excess waits onto same-engine NoOps committed just
# ahead of the instruction.
import bass_rust as _bass_rust

_MAX_WAITS = 1
_orig_commit = tile.TileContext._commit_instruction
_wsplit_counter = [0]


def _commit_instruction_split(self, inst, lazy_reg_writes=True):
    si = inst.sync_info
    if (
        si is not None
        and si.on_wait
        and len(si.on_wait) > _MAX_WAITS
        and inst.engine != mybir.EngineType.Unassigned
    ):
        waits = list(si.on_wait)
        keep = waits[-_MAX_WAITS:]
        extra = waits[:-_MAX_WAITS]
        for i in range(0, len(extra), _MAX_WAITS):
            _wsplit_counter[0] += 1
            nop = _bass_rust.InstNoOp(
                name=f"wsplit-{_wsplit_counter[0]}", ins=[], outs=[]
            )
            nop.engine = inst.engine
            nop.sync_info = _bass_rust.SyncInfo(
                on_wait=extra[i:i + _MAX_WAITS], on_update=[]
            )
            self._add_instruction(nop)
        inst.sync_info = _bass_rust.SyncInfo(
            on_wait=keep, on_update=list(si.on_update)
        )
    return _orig_commit(self, inst, lazy_reg_writes)


tile.TileContext._commit_instruction = _commit_instruction_split
# ---------------------------------------------------------------------------

B, S, D, H = 4, 4096, 2048, 512
EPS = 1e-5
NCORES = 8
TOK = B * S // NCORES  # tokens per core
P = 128
NT = TOK // P          # 16 token tiles per core
KC = D // P            # 16 contraction chunks for down-proj
HC = H // P            # 4 contraction chunks for up-proj
LAG2 = 3               # a2 trails a1 by this many tiles
LAG3 = 4               # b trails a1 by this many tiles

F32 = mybir.dt.float32
BF16 = mybir.dt.bfloat16
FP8 = mybir.dt.float8e4


def build_nc():
    nc = bass.Bass("TRN2", target_bir_lowering=False, debug=False, num_devices=NCORES)

    x_ext = nc.declare_dram_parameter("x", [TOK, D], F32, isOutput=False)
    wdT_ext = nc.declare_dram_parameter("wdT", [P, KC, H], FP8, isOutput=False)
    wuT_ext = nc.declare_dram_parameter("wuT", [P, HC, D], BF16, isOutput=False)
    seedC_ext = nc.declare_dram_parameter("seedC", [2, H], BF16, isOutput=False)
    id128_ext = nc.declare_dram_parameter("id128", [P, P], BF16, isOutput=False)
    id128f8_ext = nc.declare_dram_parameter("id128f8", [P, P], FP8, isOutput=False)
    bu_ext = nc.declare_dram_parameter("bu", [1, D], BF16, isOutput=False)
    out_ext = nc.declare_dram_parameter("out", [TOK, D], F32, isOutput=True)

    with tile.TileContext(nc) as tc:
        with (
            tc.tile_pool(name="singles", bufs=1) as singles,
            tc.tile_pool(name="xp", bufs=6) as xp,
            tc.tile_pool(name="statp", bufs=12) as statp,
            tc.tile_pool(name="yp", bufs=3) as yp,
            tc.tile_pool(name="ytp", bufs=5) as ytp,
            tc.tile_pool(name="rp", bufs=3) as rp,
            tc.tile_pool(name="rtp", bufs=4) as rtp,
            tc.tile_pool(name="op", bufs=3) as op,
            tc.tile_pool(name="pdp", bufs=2, space="PSUM") as pdp,
            tc.tile_pool(name="pup", bufs=3, space="PSUM") as pup,
            tc.tile_pool(name="ptY", bufs=2, space="PSUM") as ptY,
            tc.tile_pool(name="ptR", bufs=1, space="PSUM") as ptR,
        ):
            # -------- persistent tiles ------------------------------------
            # DMA traffic is segregated by class: bulk HBM transfers (x-in,
            # out, wdT) ride the GpSimd SWDGE ring; the latency-critical
            # xbar transposes get the 8-semaphore HWDGE ring essentially to
            # themselves (only these few small/fast startup loads precede
            # them there), so no transpose ever chains behind a 1MB x
            # transfer through a recycled completion semaphore.
            bu_row = singles.tile([1, D], BF16)
            nc.scalar.dma_start(bu_row[:], bu_ext[:])
            seedC = singles.tile([2, H], BF16)
            nc.scalar.dma_start(seedC[:], seedC_ext[:])
            id128 = singles.tile([P, P], BF16)
            nc.scalar.dma_start(id128[:], id128_ext[:])
            id128f8 = singles.tile([P, P], FP8)
            nc.scalar.dma_start(id128f8[:], id128f8_ext[:])
            wuT = singles.tile([P, HC, D], BF16)  # loaded after wdT, below
            wdT = singles.tile([P, KC, H], FP8)   # loaded after a1(0), below
            ones_row = singles.tile([1, P], BF16)
            nc.vector.memset(ones_row[:], 1.0)
            epst = singles.tile([P, 1], F32)
            nc.vector.memset(epst[:], EPS)

            # Broadcast b_up across partitions ON DEVICE (rank-1 K=1 matmuls
            # + PSUM->SBUF copies) instead of a descriptor-heavy duplicated
            # DRAM read: the PE is idle at startup, the DMA ring is not.
            bu_bc = singles.tile([P, D], F32)
            for q in range(4):
                pq = pup.tile([P, 512], F32)
                nc.tensor.matmul(pq[:], ones_row[:],
                                 bu_row[:, q * 512:(q + 1) * 512],
                                 start=True, stop=True)
                nc.scalar.copy(bu_bc[:, q * 512:(q + 1) * 512], pq[:])

            def phase_a1(t):
                """Load x; cast+transpose feed the PE; stats in parallel."""
                x_sb = xp.tile([P, D], F32)
                y_sb = yp.tile([P, D], BF16)
                yT = ytp.tile([P, KC, P], FP8)
                # x-in must ride the (fast) scalar HWDGE queue: SWDGE
                # transfer bandwidth can't keep the feed ahead of the PE.
                nc.scalar.dma_start(x_sb[:], x_ext[t * P:(t + 1) * P, :])
                # Priority keeps the cast ahead of bulk scalar work, but a2's
                # relu/rdT-copy (same priority class, emitted earlier in the
                # iteration) stay in front of it in ScalarE's FIFO.
                with tc.high_priority():
                    nc.scalar.activation(
                        y_sb[:], x_sb[:],
                        mybir.ActivationFunctionType.Identity,
                    )
                # Transpose y on the PE itself (128 cycles/slab against the
                # identity): the PE paces its own input, so no transpose can
                # ever chain behind a bulk transfer through the recycled DMA
                # completion semaphores. Slabs stage in PSUM (8 per bank,
                # each slab its own start/stop write) and copy back to SBUF
                # on ScalarE (group 0) / DVE (group 1).
                # (transposes run in bf16 -- fp8 PE-transpose needs a
                # stride-2 output -- and the PSUM->SBUF copy casts to fp8)
                for g in range(2):
                    pt = ptY.tile([P, 8, P], BF16)
                    for j in range(8):
                        nc.tensor.transpose(
                            pt[:, j, :],
                            y_sb[:, (8 * g + j) * P:(8 * g + j + 1) * P],
                            id128[:])
                    if g == 0:
                        nc.scalar.copy(yT[:, 0:8, :], pt[:])
                    else:
                        nc.vector.tensor_scalar_add(yT[:, 8:16, :], pt[:], 0.0)

                # LN stats: mu/var on DVE, std=sqrt(var+eps), istd=1/std.
                # Only consumed by the seed matmul / ReLU ~2 phases later.
                st = statp.tile([P, 4, 6], F32)
                for i in range(4):
                    nc.vector.bn_stats(st[:, i, :], x_sb[:, i * 512:(i + 1) * 512])
                mv = statp.tile([P, 2], F32)
                nc.vector.bn_aggr(mv[:], st[:])
                std = statp.tile([P, 1], F32)
                nc.scalar.activation(
                    std[:], mv[:, 1:2], mybir.ActivationFunctionType.Sqrt,
                    bias=epst[:], scale=1.0,
                )
                istd = statp.tile([P, 1], F32)
                nc.vector.reciprocal(istd[:], std[:])
                # sm = [std, -mu] as bf16 columns; row-ified in a2 via a tiny
                # PE transpose (the stats have ~3 tiles of slack until then).
                sm = statp.tile([P, 2], BF16)
                nc.scalar.copy(sm[:, 0:1], std[:])
                nc.vector.tensor_scalar(
                    sm[:, 1:2], mv[:, 0:1], -1.0, None, mybir.AluOpType.mult)

                # fold b_up into the residual on GpSimd: x <- x + b_up (after
                # the cast and bn_stats consumed raw x).
                nc.gpsimd.tensor_add(x_sb[:], x_sb[:], bu_bc[:])
                return x_sb, yT, sm, istd

            def phase_a2(t, x_sb, yT, sm, istd):
                """Down-proj on raw xT + K=2 LN seed, relu(istd*psum), rdT."""
                # One PSUM bank stages both the [std;-mu] row pair (slab 4)
                # and the 4 relu'd-down slabs (slabs 0-3) for this tile.
                pt = ptR.tile([P, 5, P], BF16)
                nc.tensor.transpose(pt[0:2, 4, :], sm[:], id128[:])
                srow = statp.tile([2, P], BF16)
                with tc.high_priority():
                    nc.scalar.copy(srow[:], pt[0:2, 4, :])

                # fp8e4m3 DoubleRow: two K=128 slabs per instruction at 2x
                # rate; yT/wdT's [P, KC, free] layout is already the required
                # [K, 2, free] sub-slab form.
                pd = pdp.tile([P, H], F32)
                for k in range(0, KC, 2):
                    nc.tensor.matmul(pd[:], yT[:, k:k + 2, :], wdT[:, k:k + 2, :],
                                     start=(k == 0), stop=False,
                                     perf_mode=mybir.MatmulPerfMode.DoubleRow)
                # LN correction seed: [std;-mu]^T @ [b_eff;w1], last so the
                # stats chain has the whole down-proj's duration of slack.
                nc.tensor.matmul(pd[:], srow[:], seedC[:], start=False, stop=True)

                rd = rp.tile([P, H], BF16)
                with tc.high_priority():
                    nc.scalar.activation(rd[:], pd[:],
                                         mybir.ActivationFunctionType.Relu,
                                         scale=istd[:])
                for c in range(HC):
                    nc.tensor.transpose(pt[:, c, :], rd[:, c * P:(c + 1) * P],
                                        id128[:])
                rdT = rtp.tile([P, HC, P], BF16)
                with tc.high_priority():
                    nc.scalar.copy(rdT[:], pt[:, 0:HC, :])
                return x_sb, rdT

            def phase_b(t, x_sb, rdT):
                """Up-proj + residual add (x already carries b_up) + store."""
                o_sb = op.tile([P, D], F32)
                for q in range(4):
                    pq = pup.tile([P, 512], F32)
                    n0 = q * 512
                    for c in range(HC):
                        nc.tensor.matmul(
                            pq[:], rdT[:, c, :], wuT[:, c, n0:n0 + 512],
                            start=(c == 0), stop=(c == HC - 1),
                        )
                    sl = slice(n0, n0 + 512)
                    nc.vector.tensor_add(o_sb[:, sl], pq[:], x_sb[:, sl])

                if t == NT - 1:
                    # split the last store across both queues: it is the
                    # only transfer left on the drain path.
                    nc.gpsimd.dma_start(out_ext[t * P:(t + 1) * P, 0:1024],
                                        o_sb[:, 0:1024])
                    nc.scalar.dma_start(out_ext[t * P:(t + 1) * P, 1024:D],
                                        o_sb[:, 1024:D])
                else:
                    nc.gpsimd.dma_start(out_ext[t * P:(t + 1) * P, :], o_sb[:])

            # staggered software pipeline: a1 runs LAG2 tiles ahead of a2 and
            # LAG3 ahead of b. Tile 0's x-load is emitted BEFORE the bulk
            # wdT loads so it leads the SWDGE queue; the k-chunked wdT lands
            # slab-by-slab just ahead of the first down-proj matmuls.
            h1, h2_ = {}, {}
            h1[0] = phase_a1(0)
            for k in range(0, KC, 4):
                nc.scalar.dma_start(wdT[:, k:k + 4, :], wdT_ext[:, k:k + 4, :])
            nc.scalar.dma_start(wuT[:, 0:2, :], wuT_ext[:, 0:2, :])
            nc.scalar.dma_start(wuT[:, 2:4, :], wuT_ext[:, 2:4, :])
            for t in range(1, NT + LAG3):
                if LAG2 <= t < NT + LAG2:
                    h2_[t - LAG2] = phase_a2(t - LAG2, *h1.pop(t - LAG2))
                if LAG3 <= t:
                    phase_b(t - LAG3, *h2_.pop(t - LAG3))
                if t < NT:
                    h1[t] = phase_a1(t)

    return nc


_NC_CACHE = None


def _get_nc():
    global _NC_CACHE
    if _NC_CACHE is None:
        _NC_CACHE = build_nc()
    return _NC_CACHE


def make_in_maps(x, ln_gamma, ln_beta, W_down, b_down, W_up, b_up):
    x2d = np.ascontiguousarray(np.asarray(x, dtype=np.float32).reshape(B * S, D))

    # Fold LN affine (gamma/beta) into the down projection exactly:
    #   W_down @ (yhat*gamma + beta) = (W_down*gamma) @ yhat + W_down @ beta
    Wd = np.asarray(W_down, dtype=np.float64)
    gamma = np.asarray(ln_gamma, dtype=np.float64)
    beta = np.asarray(ln_beta, dtype=np.float64)
    wd_eff = Wd * gamma[None, :]
    bd_eff = np.asarray(b_down, dtype=np.float64) + Wd @ beta
    w1 = wd_eff.sum(axis=1)  # W_eff @ ones_D, for the deferred-LN correction

    bf = ml_dtypes.bfloat16
    f8 = ml_dtypes.float8_e4m3fn
    wdT_host = np.ascontiguousarray(
        wd_eff.T.reshape(KC, P, H).transpose(1, 0, 2)).astype(f8)
    wuT_host = np.ascontiguousarray(
        np.asarray(W_up, dtype=np.float64).T.reshape(HC, P, D).transpose(1, 0, 2)
    ).astype(bf)
    seedC_host = np.ascontiguousarray(
        np.stack([bd_eff, w1]).reshape(2, H)).astype(bf)
    id128_host = np.eye(P, dtype=np.float32).astype(bf)
    id128f8_host = np.eye(P, dtype=np.float32).astype(f8)
    bu_host = np.ascontiguousarray(
        np.asarray(b_up, dtype=np.float32).reshape(1, D)).astype(bf)

    in_maps = []
    for i in range(NCORES):
        in_maps.append({
            "x": x2d[i * TOK:(i + 1) * TOK],
            "wdT": wdT_host,
            "wuT": wuT_host,
            "seedC": seedC_host,
            "id128": id128_host,
            "id128f8": id128f8_host,
            "bu": bu_host,
        })
    return in_maps


def gather_out(results):
    return np.concatenate(
        [np.asarray(results[i]["out"], dtype=np.float32) for i in range(NCORES)],
        axis=0,
    ).reshape(B, S, D)


def kernel(x, ln_gamma, ln_beta, W_down, b_down, W_up, b_up):
    nc = _get_nc()
    in_maps = make_in_maps(x, ln_gamma, ln_beta, W_down, b_down, W_up, b_up)
    res = run_bass_kernel_spmd(nc, in_maps, core_ids=list(range(NCORES)))
    return gather_out(res.results)
